# revision 3
# baseline (speedup 1.0000x reference)
"""Trainium2 Bass kernel for FAVOR+ (Performer) linear attention.

Problem: nn_Attention_3994319585958
  x [8, 4096, 768], Wqkv [2304, 768], w [12, 384, 64], Wp [768, 768], bp [768]
  qkv projection -> positive random features exp(w^T y - |y|^2/2)/sqrt(m)
  -> linear attention (kptv / D normalization) -> (faithful no-transpose
  reshape) -> output projection.

Sharding: data-parallel over batch B=8, one batch element per NeuronCore.
All matmuls fp32 (K=128 contraction everywhere). Per-core pipeline:

  Pass K (x2 head-blocks of 6): kT feature-major + v token-major from xT;
    staging [k; k^2] (parity-flipped for odd heads); kp = exp(W2-augmented
    matmul - 0.5*ln(m)) token-major; kptv'[65,384] (v + ones column ->
    kptv rows + ksum row) accumulated PSUM-per-tile -> SBUF.
  kptv transpose: PE transpose -> kptvT [m, 65] per head.
  Pass Q: qT feature-major; staging; qp^T feature-major = exp(W2 lhsT
    matmul); out' token-major [128,65] = qp^T-chunks @ kptvT (col 64 = D);
    y = numer * recip(D) -> Z (DRAM, Yflat layout = faithful reshape).
  Pass P: yR token-major from Z; PE transpose -> yR^T; projection @ Wp^T;
    DMA PSUM -> out. bp added on host (zeros for this problem's inputs).
"""

import math
import os

import numpy as np

from concourse import bacc
import concourse.mybir as mybir
import concourse.tile as tile
from concourse.bass_utils import run_bass_kernel_spmd
from concourse.masks import make_identity

P = 128
H = 12
B = 8
N = 4096
C = 768
HD = 64
M = 384
NT = 512                 # token tile (pass K/Q)
NTILES = N // NT         # 8
NCH = NT // P            # 4 chunks per tile
CO = C // P              # 6 c-chunks
EXP_BIAS = -0.5 * math.log(M)   # fold 1/sqrt(m) into exp

_CACHE = {}


def _install_trace_hook():
    """antenv.axon_hooks shim so trace=True works under axon (test.py only)."""
    import sys
    import types
    try:
        import antenv.axon_hooks  # noqa: F401
        return True
    except ImportError:
        pass
    try:
        sys.path.insert(0, "/root/.axon_site")
        from trn_agent_boot.trn_boot import _ntff_profile_via_ctypes
        hook = _ntff_profile_via_ctypes("/opt/axon/libaxon_pjrt.so")
        mod = types.ModuleType("antenv.axon_hooks")
        mod.get_axon_ntff_profile_hook = lambda: hook
        mod.set_axon_ntff_profile_hook = lambda h: None
        sys.modules["antenv.axon_hooks"] = mod
        import antenv
        antenv.axon_hooks = mod
        return True
    except Exception:
        return False


def _build():
    if "nc" in _CACHE:
        return _CACHE["nc"]

    nc = bacc.Bacc(None, target_bir_lowering=False)
    f32 = mybir.dt.float32

    xt_d = nc.dram_tensor("xt", [CO, P, N], f32, kind="ExternalInput")
    wqkv_d = nc.dram_tensor("wqkvt", [CO, P, 3 * C], f32, kind="ExternalInput")
    w2_d = nc.dram_tensor("w2", [P, H, M], f32, kind="ExternalInput")
    wp_d = nc.dram_tensor("wpt", [CO, P, C], f32, kind="ExternalInput")
    out_d = nc.dram_tensor("out", [N, C], f32, kind="ExternalOutput")

    with tile.TileContext(nc) as tc:
        with tc.tile_pool(name="consts", bufs=1) as consts, \
             tc.tile_pool(name="acc", bufs=1) as accp, \
             tc.tile_pool(name="dram", bufs=1, space="DRAM") as dramp, \
             tc.tile_pool(name="psA", bufs=2, space="PSUM") as psA, \
             tc.tile_pool(name="psB", bufs=2, space="PSUM") as psB, \
             tc.tile_pool(name="psC", bufs=2, space="PSUM") as psC:

            ident = consts.tile([P, P], f32, tag="ident")
            make_identity(nc, ident[:])
            cbias = consts.tile([P, 1], f32, tag="cbias")
            nc.gpsimd.memset(cbias[:], EXP_BIAS)
            w2_t = consts.tile([P, H, M], f32, tag="w2_t")
            nc.sync.dma_start(w2_t[:], w2_d.ap())

            # kptv accumulators: [d'(65, zero-padded to 128), head, m]
            kacc = accp.tile([P, H, M], f32, tag="kacc")
            nc.gpsimd.memset(kacc[:], 0.0)
            # transposed kptv: [m(128-chunk), head, m-chunk, d'(65)]
            kptvT = accp.tile([P, H, 3, 65], f32, tag="kptvT")

            z = dramp.tile([H, N // P, P, HD], f32, name="zscratch")

            # ---------------- Pass K: two head-blocks ----------------
            for hb in range(2):
                with tc.tile_pool(name="wkv", bufs=1) as wkvp, \
                     tc.tile_pool(name="xp", bufs=2) as xp, \
                     tc.tile_pool(name="stag", bufs=8) as stagp, \
                     tc.tile_pool(name="kpp", bufs=3) as kpp, \
                     tc.tile_pool(name="vtp", bufs=5) as vtp:

                    # k+v weight columns for this block:
                    #   k cols: 768 + hb*384 .. +384 ; v cols: 1536 + hb*384 .. +384
                    wkv = wkvp.tile([P, CO, 2, M], f32, tag="wkv")
                    nc.sync.dma_start(
                        wkv[:, :, 0, :],
                        wqkv_d.ap().rearrange("a p f -> p a f")[
                            :, :, C + hb * M: C + hb * M + M],
                    )
                    nc.sync.dma_start(
                        wkv[:, :, 1, :],
                        wqkv_d.ap().rearrange("a p f -> p a f")[
                            :, :, 2 * C + hb * M: 2 * C + hb * M + M],
                    )

                    for t in range(NTILES):
                        xt = xp.tile([P, CO, NT], f32, tag="xt")
                        nc.sync.dma_start(
                            xt[:],
                            xt_d.ap().rearrange("a p n -> p a n")[
                                :, :, t * NT:(t + 1) * NT],
                        )

                        # kT feature-major for 3 head-pairs + staging
                        stags = []
                        for fc in range(3):
                            psk = psA.tile([P, NT], f32, tag="psA", name=f"psk{hb}{t}{fc}")
                            for cc in range(CO):
                                nc.tensor.matmul(
                                    psk[:],
                                    wkv[:, cc, 0, fc * P:(fc + 1) * P],
                                    xt[:, cc, :],
                                    start=(cc == 0), stop=(cc == CO - 1),
                                )
                            s0 = stagp.tile([P, NT], f32, tag="stag", name=f"s0_{hb}{t}{fc}")
                            s1 = stagp.tile([P, NT], f32, tag="stag", name=f"s1_{hb}{t}{fc}")
                            # even head: [k; k^2]; odd head: [k^2; k]
                            nc.vector.tensor_copy(s0[0:64, :], psk[0:64, :])
                            nc.vector.tensor_copy(s1[64:128, :], psk[64:128, :])
                            nc.vector.tensor_tensor(
                                s0[64:128, :], s0[0:64, :], s0[0:64, :],
                                mybir.AluOpType.mult)
                            nc.vector.tensor_tensor(
                                s1[0:64, :], s1[64:128, :], s1[64:128, :],
                                mybir.AluOpType.mult)
                            stags.append((s0, s1))

                        # v token-major + ones column
                        vts = []
                        for ch in range(NCH):
                            psv = psA.tile([P, NT], f32, tag="psA", name=f"psv{hb}{t}{ch}")
                            for cc in range(CO):
                                nc.tensor.matmul(
                                    psv[:, 0:M],
                                    xt[:, cc, ch * P:(ch + 1) * P],
                                    wkv[:, cc, 1, :],
                                    start=(cc == 0), stop=(cc == CO - 1),
                                )
                            vt = vtp.tile([P, 6, 65], f32, tag="vt", name=f"vt{hb}{t}{ch}")
                            nc.vector.tensor_copy(
                                vt[:, :, 0:64],
                                psv[:, 0:M].rearrange("p (g d) -> p g d", g=6))
                            nc.vector.memset(vt[:, :, 64:65], 1.0)
                            vts.append(vt)

                        # per head: kp = exp(...), kptv accumulate
                        for j in range(6):
                            h = hb * 6 + j
                            stag = stags[j // 2][j % 2]
                            pkv = psC.tile([P, M], f32, tag="psC", name=f"pkv{hb}{t}{j}")
                            for ch in range(NCH):
                                pkp = psB.tile([P, M], f32, tag="psB", name=f"pkp{hb}{t}{j}{ch}")
                                nc.tensor.matmul(
                                    pkp[:],
                                    stag[:, ch * P:(ch + 1) * P],
                                    w2_t[:, h, :],
                                    start=True, stop=True,
                                )
                                kp = kpp.tile([P, M], f32, tag="kp", name=f"kp{hb}{t}{j}{ch}")
                                nc.scalar.activation(
                                    kp[:], pkp[:],
                                    mybir.ActivationFunctionType.Exp,
                                    bias=cbias[:], scale=1.0)
                                nc.tensor.matmul(
                                    pkv[0:65, :],
                                    vts[ch][:, j, :],
                                    kp[:],
                                    start=(ch == 0), stop=(ch == NCH - 1),
                                )
                            nc.vector.tensor_tensor(
                                kacc[0:65, h, :], pkv[0:65, :], kacc[0:65, h, :],
                                mybir.AluOpType.add)

            # ---------------- kptv transposes ----------------
            for h in range(H):
                for mc in range(3):
                    pst = psA.tile([P, P], f32, tag="psA", name=f"pst{h}{mc}")
                    nc.tensor.transpose(
                        pst[:], kacc[:, h, mc * P:(mc + 1) * P], ident[:])
                    nc.vector.tensor_copy(kptvT[:, h, mc, :], pst[:, 0:65])

            # ---------------- Pass Q ----------------
            with tc.tile_pool(name="wq", bufs=1) as wqp, \
                 tc.tile_pool(name="xp2", bufs=2) as xp2, \
                 tc.tile_pool(name="stag2", bufs=14) as stagp2, \
                 tc.tile_pool(name="qpt", bufs=2) as qptp, \
                 tc.tile_pool(name="yp", bufs=4) as ypool, \
                 tc.tile_pool(name="rdp", bufs=4) as rdp:

                wq = wqp.tile([P, CO, C], f32, tag="wq")
                nc.sync.dma_start(
                    wq[:], wqkv_d.ap().rearrange("a p f -> p a f")[:, :, 0:C])

                for t in range(NTILES):
                    xt = xp2.tile([P, CO, NT], f32, tag="xt2")
                    nc.sync.dma_start(
                        xt[:],
                        xt_d.ap().rearrange("a p n -> p a n")[
                            :, :, t * NT:(t + 1) * NT],
                    )

                    stags = []
                    for fc in range(CO):
                        psq = psA.tile([P, NT], f32, tag="psA", name=f"psq{t}{fc}")
                        for cc in range(CO):
                            nc.tensor.matmul(
                                psq[:],
                                wq[:, cc, fc * P:(fc + 1) * P],
                                xt[:, cc, :],
                                start=(cc == 0), stop=(cc == CO - 1),
                            )
                        s0 = stagp2.tile([P, NT], f32, tag="stag2", name=f"q0_{t}{fc}")
                        s1 = stagp2.tile([P, NT], f32, tag="stag2", name=f"q1_{t}{fc}")
                        nc.vector.tensor_copy(s0[0:64, :], psq[0:64, :])
                        nc.vector.tensor_copy(s1[64:128, :], psq[64:128, :])
                        nc.vector.tensor_tensor(
                            s0[64:128, :], s0[0:64, :], s0[0:64, :],
                            mybir.AluOpType.mult)
                        nc.vector.tensor_tensor(
                            s1[0:64, :], s1[64:128, :], s1[64:128, :],
                            mybir.AluOpType.mult)
                        stags.append((s0, s1))

                    for h in range(H):
                        stag = stags[h // 2][h % 2]
                        qpt = qptp.tile([P, 3, NT], f32, tag="qpt", name=f"qpt{t}{h}")
                        for mc in range(3):
                            pqp = psB.tile([P, NT], f32, tag="psB", name=f"pqp{t}{h}{mc}")
                            nc.tensor.matmul(
                                pqp[:],
                                w2_t[:, h, mc * P:(mc + 1) * P],
                                stag[:],
                                start=True, stop=True,
                            )
                            nc.scalar.activation(
                                qpt[:, mc, :], pqp[:],
                                mybir.ActivationFunctionType.Exp,
                                bias=cbias[:], scale=1.0)

                        for ch in range(NCH):
                            po = psC.tile([P, 65], f32, tag="psC", name=f"po{t}{h}{ch}")
                            for mc in range(3):
                                nc.tensor.matmul(
                                    po[:],
                                    qpt[:, mc, ch * P:(ch + 1) * P],
                                    kptvT[:, h, mc, :],
                                    start=(mc == 0), stop=(mc == 2),
                                )
                            rd = rdp.tile([P, 1], f32, tag="rd", name=f"rd{t}{h}{ch}")
                            nc.vector.reciprocal(rd[:], po[:, 64:65])
                            y = ypool.tile([P, HD], f32, tag="y", name=f"y{t}{h}{ch}")
                            nc.vector.tensor_scalar_mul(y[:], po[:, 0:64], rd[:])
                            nc.sync.dma_start(z[h, t * NCH + ch], y[:])

            # ---------------- Pass P: projection ----------------
            zflat = z.rearrange("h t p d -> (h t p d)").rearrange(
                "(n c) -> n c", c=C)
            with tc.tile_pool(name="wpp", bufs=1) as wpp, \
                 tc.tile_pool(name="yrp", bufs=2) as yrp, \
                 tc.tile_pool(name="outp", bufs=3) as outp, \
                 tc.tile_pool(name="yrtp", bufs=2) as yrtp:

                wp_t = wpp.tile([P, CO, C], f32, tag="wp_t")
                nc.sync.dma_start(
                    wp_t[:], wp_d.ap().rearrange("a p f -> p a f"))

                for tc2 in range(N // P):
                    yr = yrp.tile([P, C], f32, tag="yr", name=f"yr{tc2}")
                    nc.sync.dma_start(
                        yr[:], zflat[tc2 * P:(tc2 + 1) * P, :])
                    yrt = yrtp.tile([P, CO, P], f32, tag="yrt", name=f"yrt{tc2}")
                    for cc in range(CO):
                        pst = psA.tile([P, P], f32, tag="psA", name=f"pp{tc2}{cc}")
                        nc.tensor.transpose(
                            pst[:], yr[:, cc * P:(cc + 1) * P], ident[:])
                        nc.vector.tensor_copy(yrt[:, cc, :], pst[:])
                    for nch in range(2):
                        pf = psB.tile([P, M], f32, tag="psB", name=f"pf{tc2}{nch}")
                        for cc in range(CO):
                            nc.tensor.matmul(
                                pf[:],
                                yrt[:, cc, :],
                                wp_t[:, cc, nch * M:(nch + 1) * M],
                                start=(cc == 0), stop=(cc == CO - 1),
                            )
                        ot = outp.tile([P, M], f32, tag="ot", name=f"ot{tc2}{nch}")
                        nc.vector.tensor_copy(ot[:], pf[:])
                        nc.sync.dma_start(
                            out_d.ap()[tc2 * P:(tc2 + 1) * P,
                                       nch * M:(nch + 1) * M],
                            ot[:])

    nc.compile()
    _CACHE["nc"] = nc
    return nc


def _prep_inputs(x, Wqkv, w, Wp):
    """Host-side layout prep -> per-core input maps."""
    wqkvt = np.ascontiguousarray(Wqkv.T).reshape(CO, P, 3 * C)
    wpt = np.ascontiguousarray(Wp.T).reshape(CO, P, C)
    w2 = np.empty((P, H, M), dtype=np.float32)
    for h in range(H):
        wt = np.ascontiguousarray(w[h].T)          # [64, 384]
        if h % 2 == 0:
            w2[0:64, h, :] = wt
            w2[64:128, h, :] = -0.5
        else:
            w2[0:64, h, :] = -0.5
            w2[64:128, h, :] = wt
    in_maps = []
    for b in range(B):
        xt = np.ascontiguousarray(x[b].T).reshape(CO, P, N)
        in_maps.append({"xt": xt, "wqkvt": wqkvt, "w2": w2, "wpt": wpt})
    return in_maps


def run(x, Wqkv, w, Wp, bp, trace=False):
    """Run the kernel on 8 cores; returns (out [B,N,C], BassKernelResults)."""
    x = np.asarray(x, dtype=np.float32)
    Wqkv = np.asarray(Wqkv, dtype=np.float32)
    w = np.asarray(w, dtype=np.float32)
    Wp = np.asarray(Wp, dtype=np.float32)
    bp = np.asarray(bp, dtype=np.float32)

    if trace:
        _install_trace_hook()
    nc = _build()
    in_maps = _prep_inputs(x, Wqkv, w, Wp)
    res = run_bass_kernel_spmd(
        nc, in_maps, core_ids=list(range(B)), trace=trace)
    out = np.stack([res.results[b]["out"] for b in range(B)], axis=0)
    out = out + bp[None, None, :]
    return out, res


def kernel(x, Wqkv, w, Wp, bp):
    out, _ = run(x, Wqkv, w, Wp, bp, trace=False)
    return out


if __name__ == "__main__":
    rng = np.random.default_rng(0)
    xs = rng.standard_normal((B, N, C), dtype=np.float32)
    Wq = (rng.standard_normal((3 * C, C), dtype=np.float32) / math.sqrt(C))
    ws = rng.standard_normal((H, M, HD), dtype=np.float32)
    ws = ws / np.linalg.norm(ws, axis=-1, keepdims=True) * math.sqrt(M)
    Wpp = (rng.standard_normal((C, C), dtype=np.float32) / math.sqrt(C))
    bpp = np.zeros((C,), dtype=np.float32)
    o = kernel(xs, Wq, ws, Wpp, bpp)
    print("out", o.shape, o.dtype, np.isnan(o).sum(), np.isinf(o).sum())


# revision 5
# speedup vs baseline: 1.1977x; 1.1977x over previous
"""Trainium2 Bass kernel for FAVOR+ (Performer) linear attention.

Problem: nn_Attention_3994319585958
  x [8, 4096, 768], Wqkv [2304, 768], w [12, 384, 64], Wp [768, 768], bp [768]
  qkv projection -> positive random features exp(w^T y - |y|^2/2)/sqrt(m)
  -> linear attention (kptv / D normalization) -> (faithful no-transpose
  reshape) -> output projection.

Sharding: data-parallel over batch B=8, one batch element per NeuronCore.
All matmuls fp32 (K=128 contraction everywhere). Per-core pipeline:

  Pass K (x2 head-blocks of 6): kT feature-major + v token-major from xT;
    staging [k; k^2] (parity-flipped for odd heads); kp = exp(W2-augmented
    matmul - 0.5*ln(m)) token-major; kptv'[65,384] (v + ones column ->
    kptv rows + ksum row) accumulated PSUM-per-tile -> SBUF.
  kptv transpose: PE transpose -> kptvT [m, 65] per head.
  Pass Q: qT feature-major; staging; qp^T feature-major = exp(W2 lhsT
    matmul); out' token-major [128,65] = qp^T-chunks @ kptvT (col 64 = D);
    y = numer * recip(D) -> Z (DRAM, Yflat layout = faithful reshape).
  Pass P: yR token-major from Z; PE transpose -> yR^T; projection @ Wp^T;
    DMA PSUM -> out. bp added on host (zeros for this problem's inputs).
"""

import math
import os

import numpy as np

from concourse import bacc
import concourse.mybir as mybir
import concourse.tile as tile
from concourse.bass_utils import run_bass_kernel_spmd
from concourse.masks import make_identity

P = 128
H = 12
B = 8
N = 4096
C = 768
HD = 64
M = 384
NT = 512                 # token tile (pass K/Q)
NTILES = N // NT         # 8
NCH = NT // P            # 4 chunks per tile
CO = C // P              # 6 c-chunks
EXP_BIAS = -0.5 * math.log(M)   # fold 1/sqrt(m) into exp

_CACHE = {}


def _install_trace_hook():
    """antenv.axon_hooks shim so trace=True works under axon (test.py only)."""
    import sys
    import types
    try:
        import antenv.axon_hooks  # noqa: F401
        return True
    except ImportError:
        pass
    try:
        sys.path.insert(0, "/root/.axon_site")
        from trn_agent_boot.trn_boot import _ntff_profile_via_ctypes
        hook = _ntff_profile_via_ctypes("/opt/axon/libaxon_pjrt.so")
        mod = types.ModuleType("antenv.axon_hooks")
        mod.get_axon_ntff_profile_hook = lambda: hook
        mod.set_axon_ntff_profile_hook = lambda h: None
        sys.modules["antenv.axon_hooks"] = mod
        import antenv
        antenv.axon_hooks = mod
        return True
    except Exception:
        return False


def _build():
    if "nc" in _CACHE:
        return _CACHE["nc"]

    nc = bacc.Bacc(None, target_bir_lowering=False)
    f32 = mybir.dt.float32

    xt_d = nc.dram_tensor("xt", [CO, P, N], f32, kind="ExternalInput")
    wqkv_d = nc.dram_tensor("wqkvt", [CO, P, 3 * C], f32, kind="ExternalInput")
    w2_d = nc.dram_tensor("w2", [P, H, M], f32, kind="ExternalInput")
    wp_d = nc.dram_tensor("wpt", [CO, P, C], f32, kind="ExternalInput")
    out_d = nc.dram_tensor("out", [N, C], f32, kind="ExternalOutput")

    with tile.TileContext(nc) as tc:
        with tc.tile_pool(name="consts", bufs=1) as consts, \
             tc.tile_pool(name="acc", bufs=1) as accp, \
             tc.tile_pool(name="dram", bufs=1, space="DRAM") as dramp, \
             tc.tile_pool(name="psA", bufs=2, space="PSUM") as psA, \
             tc.tile_pool(name="psB", bufs=3, space="PSUM") as psB, \
             tc.tile_pool(name="psC", bufs=2, space="PSUM") as psC:

            ident = consts.tile([P, P], f32, tag="ident")
            make_identity(nc, ident[:])
            cbias = consts.tile([P, 1], f32, tag="cbias")
            nc.gpsimd.memset(cbias[:], EXP_BIAS)
            w2_t = consts.tile([P, H, M], f32, tag="w2_t")
            nc.sync.dma_start(w2_t[:], w2_d.ap())

            # kptv accumulators: [d'(65, zero-padded to 128), head, m]
            kacc = accp.tile([P, H, M], f32, tag="kacc")
            nc.gpsimd.memset(kacc[:], 0.0)
            # transposed kptv: [m(128-chunk), head, m-chunk, d'(65)]
            kptvT = accp.tile([P, H, 3, 65], f32, tag="kptvT")

            z = dramp.tile([H, N // P, P, HD], f32, name="zscratch")

            # ---------------- Pass K: two head-blocks ----------------
            with tc.tile_pool(name="wkv", bufs=2) as wkvp:
              for hb in range(2):
                with tc.tile_pool(name="xp", bufs=3) as xp, \
                     tc.tile_pool(name="stag", bufs=8) as stagp, \
                     tc.tile_pool(name="kpp", bufs=3) as kpp, \
                     tc.tile_pool(name="vtp", bufs=5) as vtp:

                    # k+v weight columns for this block:
                    #   k cols: 768 + hb*384 .. +384 ; v cols: 1536 + hb*384 .. +384
                    wkv = wkvp.tile([P, CO, 2, M], f32, tag="wkv", name=f"wkv{hb}")
                    nc.sync.dma_start(
                        wkv[:, :, 0, :],
                        wqkv_d.ap().rearrange("a p f -> p a f")[
                            :, :, C + hb * M: C + hb * M + M],
                    )
                    nc.sync.dma_start(
                        wkv[:, :, 1, :],
                        wqkv_d.ap().rearrange("a p f -> p a f")[
                            :, :, 2 * C + hb * M: 2 * C + hb * M + M],
                    )

                    for t in range(NTILES):
                        xt = xp.tile([P, CO, NT], f32, tag="xt")
                        nc.sync.dma_start(
                            xt[:],
                            xt_d.ap().rearrange("a p n -> p a n")[
                                :, :, t * NT:(t + 1) * NT],
                        )

                        # kT feature-major for 3 head-pairs + staging
                        stags = []
                        for fc in range(3):
                            psk = psA.tile([P, NT], f32, tag="psA", name=f"psk{hb}{t}{fc}")
                            for cc in range(CO):
                                nc.tensor.matmul(
                                    psk[:],
                                    wkv[:, cc, 0, fc * P:(fc + 1) * P],
                                    xt[:, cc, :],
                                    start=(cc == 0), stop=(cc == CO - 1),
                                )
                            s0 = stagp.tile([P, NT], f32, tag="stag", name=f"s0_{hb}{t}{fc}")
                            s1 = stagp.tile([P, NT], f32, tag="stag", name=f"s1_{hb}{t}{fc}")
                            # even head: [k; k^2]; odd head: [k^2; k]
                            nc.vector.tensor_copy(s0[0:64, :], psk[0:64, :])
                            nc.vector.tensor_copy(s1[64:128, :], psk[64:128, :])
                            nc.vector.tensor_tensor(
                                s0[64:128, :], s0[0:64, :], s0[0:64, :],
                                mybir.AluOpType.mult)
                            nc.vector.tensor_tensor(
                                s1[0:64, :], s1[64:128, :], s1[64:128, :],
                                mybir.AluOpType.mult)
                            stags.append((s0, s1))

                        # v token-major + ones column
                        vts = []
                        for ch in range(NCH):
                            psv = psA.tile([P, NT], f32, tag="psA", name=f"psv{hb}{t}{ch}")
                            for cc in range(CO):
                                nc.tensor.matmul(
                                    psv[:, 0:M],
                                    xt[:, cc, ch * P:(ch + 1) * P],
                                    wkv[:, cc, 1, :],
                                    start=(cc == 0), stop=(cc == CO - 1),
                                )
                            vt = vtp.tile([P, 6, 65], f32, tag="vt", name=f"vt{hb}{t}{ch}")
                            nc.vector.tensor_copy(
                                vt[:, :, 0:64],
                                psv[:, 0:M].rearrange("p (g d) -> p g d", g=6))
                            nc.vector.memset(vt[:, :, 64:65], 1.0)
                            vts.append(vt)

                        # per head: kp = exp(...), kptv accumulate
                        for j in range(6):
                            h = hb * 6 + j
                            stag = stags[j // 2][j % 2]
                            pkv = psC.tile([P, M], f32, tag="psC", name=f"pkv{hb}{t}{j}")
                            for ch in range(NCH):
                                pkp = psB.tile([P, M], f32, tag="psB", name=f"pkp{hb}{t}{j}{ch}")
                                nc.tensor.matmul(
                                    pkp[:],
                                    stag[:, ch * P:(ch + 1) * P],
                                    w2_t[:, h, :],
                                    start=True, stop=True,
                                )
                                kp = kpp.tile([P, M], f32, tag="kp", name=f"kp{hb}{t}{j}{ch}")
                                nc.scalar.activation(
                                    kp[:], pkp[:],
                                    mybir.ActivationFunctionType.Exp,
                                    bias=cbias[:], scale=1.0)
                                nc.tensor.matmul(
                                    pkv[0:65, :],
                                    vts[ch][:, j, :],
                                    kp[:],
                                    start=(ch == 0), stop=(ch == NCH - 1),
                                )
                            nc.vector.tensor_tensor(
                                kacc[0:65, h, :], pkv[0:65, :], kacc[0:65, h, :],
                                mybir.AluOpType.add)

            # ---------------- kptv transposes ----------------
            for h in range(H):
                for mc in range(3):
                    pst = psA.tile([P, P], f32, tag="psA", name=f"pst{h}{mc}")
                    nc.tensor.transpose(
                        pst[:], kacc[:, h, mc * P:(mc + 1) * P], ident[:])
                    nc.vector.tensor_copy(kptvT[:, h, mc, :], pst[:, 0:65])

            # ---------------- Pass Q ----------------
            with tc.tile_pool(name="wq", bufs=1) as wqp, \
                 tc.tile_pool(name="xp2", bufs=2) as xp2, \
                 tc.tile_pool(name="stag2", bufs=14) as stagp2, \
                 tc.tile_pool(name="qpt", bufs=2) as qptp, \
                 tc.tile_pool(name="yp", bufs=4) as ypool, \
                 tc.tile_pool(name="rdp", bufs=4) as rdp:

                wq = wqp.tile([P, CO, C], f32, tag="wq")
                nc.sync.dma_start(
                    wq[:], wqkv_d.ap().rearrange("a p f -> p a f")[:, :, 0:C])

                for t in range(NTILES):
                    xt = xp2.tile([P, CO, NT], f32, tag="xt2")
                    nc.sync.dma_start(
                        xt[:],
                        xt_d.ap().rearrange("a p n -> p a n")[
                            :, :, t * NT:(t + 1) * NT],
                    )

                    stags = []
                    for fc in range(CO):
                        psq = psA.tile([P, NT], f32, tag="psA", name=f"psq{t}{fc}")
                        for cc in range(CO):
                            nc.tensor.matmul(
                                psq[:],
                                wq[:, cc, fc * P:(fc + 1) * P],
                                xt[:, cc, :],
                                start=(cc == 0), stop=(cc == CO - 1),
                            )
                        s0 = stagp2.tile([P, NT], f32, tag="stag2", name=f"q0_{t}{fc}")
                        s1 = stagp2.tile([P, NT], f32, tag="stag2", name=f"q1_{t}{fc}")
                        nc.vector.tensor_copy(s0[0:64, :], psq[0:64, :])
                        nc.vector.tensor_copy(s1[64:128, :], psq[64:128, :])
                        nc.vector.tensor_tensor(
                            s0[64:128, :], s0[0:64, :], s0[0:64, :],
                            mybir.AluOpType.mult)
                        nc.vector.tensor_tensor(
                            s1[0:64, :], s1[64:128, :], s1[64:128, :],
                            mybir.AluOpType.mult)
                        stags.append((s0, s1))

                    for h in range(H):
                        stag = stags[h // 2][h % 2]
                        qpt = qptp.tile([P, 3, NT], f32, tag="qpt", name=f"qpt{t}{h}")
                        for mc in range(3):
                            pqp = psB.tile([P, NT], f32, tag="psB", name=f"pqp{t}{h}{mc}")
                            nc.tensor.matmul(
                                pqp[:],
                                w2_t[:, h, mc * P:(mc + 1) * P],
                                stag[:],
                                start=True, stop=True,
                            )
                            nc.scalar.activation(
                                qpt[:, mc, :], pqp[:],
                                mybir.ActivationFunctionType.Exp,
                                bias=cbias[:], scale=1.0)

                        for ch in range(NCH):
                            po = psC.tile([P, 65], f32, tag="psC", name=f"po{t}{h}{ch}")
                            for mc in range(3):
                                nc.tensor.matmul(
                                    po[:],
                                    qpt[:, mc, ch * P:(ch + 1) * P],
                                    kptvT[:, h, mc, :],
                                    start=(mc == 0), stop=(mc == 2),
                                )
                            rd = rdp.tile([P, 1], f32, tag="rd", name=f"rd{t}{h}{ch}")
                            nc.vector.reciprocal(rd[:], po[:, 64:65])
                            y = ypool.tile([P, HD], f32, tag="y", name=f"y{t}{h}{ch}")
                            nc.vector.tensor_scalar_mul(y[:], po[:, 0:64], rd[:])
                            nc.sync.dma_start(z[h, t * NCH + ch], y[:])

            # ---------------- Pass P: projection ----------------
            zflat = z.rearrange("h t p d -> (h t p d)").rearrange(
                "(n c) -> n c", c=C)
            with tc.tile_pool(name="wpp", bufs=1) as wpp, \
                 tc.tile_pool(name="yrp", bufs=2) as yrp, \
                 tc.tile_pool(name="outp", bufs=3) as outp, \
                 tc.tile_pool(name="yrtp", bufs=2) as yrtp:

                wp_t = wpp.tile([P, CO, C], f32, tag="wp_t")
                nc.sync.dma_start(
                    wp_t[:], wp_d.ap().rearrange("a p f -> p a f"))

                for tc2 in range(N // P):
                    yr = yrp.tile([P, C], f32, tag="yr", name=f"yr{tc2}")
                    nc.sync.dma_start(
                        yr[:], zflat[tc2 * P:(tc2 + 1) * P, :])
                    yrt = yrtp.tile([P, CO, P], f32, tag="yrt", name=f"yrt{tc2}")
                    for cc in range(CO):
                        pst = psA.tile([P, P], f32, tag="psA", name=f"pp{tc2}{cc}")
                        nc.tensor.transpose(
                            pst[:], yr[:, cc * P:(cc + 1) * P], ident[:])
                        nc.vector.tensor_copy(yrt[:, cc, :], pst[:])
                    for nch in range(2):
                        pf = psB.tile([P, M], f32, tag="psB", name=f"pf{tc2}{nch}")
                        for cc in range(CO):
                            nc.tensor.matmul(
                                pf[:],
                                yrt[:, cc, :],
                                wp_t[:, cc, nch * M:(nch + 1) * M],
                                start=(cc == 0), stop=(cc == CO - 1),
                            )
                        ot = outp.tile([P, M], f32, tag="ot", name=f"ot{tc2}{nch}")
                        nc.vector.tensor_copy(ot[:], pf[:])
                        nc.sync.dma_start(
                            out_d.ap()[tc2 * P:(tc2 + 1) * P,
                                       nch * M:(nch + 1) * M],
                            ot[:])

    nc.compile()
    _CACHE["nc"] = nc
    return nc


def _prep_inputs(x, Wqkv, w, Wp):
    """Host-side layout prep -> per-core input maps."""
    wqkvt = np.ascontiguousarray(Wqkv.T).reshape(CO, P, 3 * C)
    wpt = np.ascontiguousarray(Wp.T).reshape(CO, P, C)
    w2 = np.empty((P, H, M), dtype=np.float32)
    for h in range(H):
        wt = np.ascontiguousarray(w[h].T)          # [64, 384]
        if h % 2 == 0:
            w2[0:64, h, :] = wt
            w2[64:128, h, :] = -0.5
        else:
            w2[0:64, h, :] = -0.5
            w2[64:128, h, :] = wt
    in_maps = []
    for b in range(B):
        xt = np.ascontiguousarray(x[b].T).reshape(CO, P, N)
        in_maps.append({"xt": xt, "wqkvt": wqkvt, "w2": w2, "wpt": wpt})
    return in_maps


def run(x, Wqkv, w, Wp, bp, trace=False):
    """Run the kernel on 8 cores; returns (out [B,N,C], BassKernelResults)."""
    x = np.asarray(x, dtype=np.float32)
    Wqkv = np.asarray(Wqkv, dtype=np.float32)
    w = np.asarray(w, dtype=np.float32)
    Wp = np.asarray(Wp, dtype=np.float32)
    bp = np.asarray(bp, dtype=np.float32)

    if trace:
        _install_trace_hook()
    nc = _build()
    in_maps = _prep_inputs(x, Wqkv, w, Wp)
    res = run_bass_kernel_spmd(
        nc, in_maps, core_ids=list(range(B)), trace=trace)
    out = np.stack([res.results[b]["out"] for b in range(B)], axis=0)
    out = out + bp[None, None, :]
    return out, res


def kernel(x, Wqkv, w, Wp, bp):
    out, _ = run(x, Wqkv, w, Wp, bp, trace=False)
    return out


if __name__ == "__main__":
    rng = np.random.default_rng(0)
    xs = rng.standard_normal((B, N, C), dtype=np.float32)
    Wq = (rng.standard_normal((3 * C, C), dtype=np.float32) / math.sqrt(C))
    ws = rng.standard_normal((H, M, HD), dtype=np.float32)
    ws = ws / np.linalg.norm(ws, axis=-1, keepdims=True) * math.sqrt(M)
    Wpp = (rng.standard_normal((C, C), dtype=np.float32) / math.sqrt(C))
    bpp = np.zeros((C,), dtype=np.float32)
    o = kernel(xs, Wq, ws, Wpp, bpp)
    print("out", o.shape, o.dtype, np.isnan(o).sum(), np.isinf(o).sum())
